# revision 37
# baseline (speedup 1.0000x reference)
"""Trainium2 Bass kernel: attention layer (B=4, S=2048, D=1024), 8 NeuronCores.

Sharding: data-parallel over (batch, query-half) -> 8 shards. Each core
computes one batch's half of the queries against that batch's full key/value
(tensor-parallel K/V splits were measured and rejected: a pairwise 4MB
AllGather costs >150us on this part, far more than the 60us of matmuls
it would save).

Per-core dataflow (all transpose-free; host pre-transposes inputs):
  QT[e,q]   = Wq projection of queries (fp32r matmuls; stays resident)
  KT chunks = Wk projection, fused with the score phase: each [e,512k]
              drain chunk is consumed as score-matmul lhsT straight from
              SBUF, so KT is never materialized or spilled
  ST[k,q]   = scores, k on partitions (fp32r, fp32 PSUM); spilled to DRAM
              except the last two k-tiles; row-max tracked on the fly by
              a DVE max chain
  softmax along k (the partition dim): 7-step DMA-shift partition-halving
    max -> m[1,q]; broadcast to [128,q] via DRAM bounce + zero-partition-
    stride reload; E = exp(ST - m) in bf16 (ST reloads land in the freed
    QT slots; subtract runs in place); l accumulated on DVE as E appears
  V[k,e]    = value projection (fp32r from the resident key tiles),
              emitted after the scores so the PE stays busy through the
              softmax reductions; spills to DRAM, reloads into freed SBUF
  O[q,e]    = (E.T @ V) matmuls (bf16); the l row-sum ones-matmuls, the
              PE-transpose of 1/l into [q,1] layout, and the scaled
              stores are threaded into the middle of the PV loop
Engine budget at ~330us: PE ~281us busy (84%), DVE ~137, ACT ~49,
gpsimd ~62, sync ~36; MFU ~72%.

float32r matmul measured on silicon: ~1 cycle/row at free-dim 512 (vs
fp32's 4) with ~1.5e-4 relative precision -- enough for the unscaled
(near-one-hot, logit std ~34) softmax here; bf16 scores would flip
argmax rows and fail (measured 2.5e-2). bf16 is fine for E and the PV
contraction. End-to-end rel err vs the fp32 reference: 2.3e-3.
"""

import numpy as np
import ml_dtypes
from contextlib import ExitStack

import concourse.bass as bass
import concourse.tile as tile
from concourse import bacc, mybir
from concourse.bass import ts
from concourse.bass_utils import run_bass_kernel_spmd

B, S, D = 4, 2048, 1024
N_CORES = 8
SQ = S // 2            # 1024 query rows per core
P = 128                # partitions
NE = D // P            # 8 e-tiles
ND = D // P            # 8 d-tiles
NK = S // P            # 16 k-tiles
NQC = SQ // P          # 8 q-chunks
F32R = mybir.dt.float32r
F32 = mybir.dt.float32
BF16 = mybir.dt.bfloat16

_NC_CACHE = {}


def _build():
    if "nc" in _NC_CACHE:
        return _NC_CACHE["nc"]
    nc = bacc.Bacc("TRN2", target_bir_lowering=False, debug=False,
                   num_devices=N_CORES)

    qT = nc.dram_tensor("qT", [D, SQ], F32R, kind="ExternalInput")
    kT = nc.dram_tensor("kT", [D, S], F32R, kind="ExternalInput")
    wqT = nc.dram_tensor("wqT", [D, D], F32R, kind="ExternalInput")
    wkT = nc.dram_tensor("wkT", [D, D], F32R, kind="ExternalInput")
    wvT = nc.dram_tensor("wvT", [D, D], F32R, kind="ExternalInput")
    out = nc.dram_tensor("out", [SQ, D], F32, kind="ExternalOutput")

    from concourse.masks import make_identity

    with tile.TileContext(nc) as tc:
        with ExitStack() as ctx:
            psum = ctx.enter_context(tc.tile_pool(name="psum", bufs=6, space="PSUM"))
            psl = ctx.enter_context(tc.tile_pool(name="psl", bufs=1, space="PSUM"))
            dram = ctx.enter_context(tc.tile_pool(name="dram", bufs=1, space="DRAM"))
            consts = ctx.enter_context(tc.tile_pool(name="consts", bufs=1))
            maxp = ctx.enter_context(tc.tile_pool(name="maxp", bufs=1))
            qtp = ctx.enter_context(tc.tile_pool(name="qtp", bufs=NE))

            id8 = consts.tile([8, 8], F32)
            make_identity(nc, id8[:])

            macc = maxp.tile([P, SQ], F32)
            stbp = ctx.enter_context(tc.tile_pool(name="stbp", bufs=2))
            tmp = maxp.tile([64, SQ], F32)
            m_bc = maxp.tile([P, SQ], F32)

            st_spill = [dram.tile([P, SQ], F32, tag="stsp", name=f"stsp{i}")
                        for i in range(NK)]
            v_spill = [dram.tile([P, D], BF16, tag="vsp", name=f"vsp{i}")
                       for i in range(NK)]

            # ---- P1: QT[e,q] projection; stays resident in SBUF -------
            # kin (the f32r key tiles) opens early so its 8MB load runs
            # on the sync queue during the P1 matmuls.
            kin_ctx = ExitStack()
            kin = kin_ctx.enter_context(tc.tile_pool(name="kin", bufs=ND))
            kts = [kin.tile([P, S], F32R, tag="kin", name=f"kin{i}")
                   for i in range(ND)]
            QTr = [qtp.tile([P, SQ], F32R, tag="qtr", name=f"qtr{i}")
                   for i in range(NE)]
            with tc.tile_pool(name="qin", bufs=ND) as qin, \
                 tc.tile_pool(name="wq", bufs=ND) as wq:
                qts = [qin.tile([P, SQ], F32R, tag="qin", name=f"qin{i}")
                       for i in range(ND)]
                wqs = [wq.tile([P, D], F32R, tag="wq", name=f"wq{i}")
                       for i in range(ND)]
                for d in range(ND):
                    nc.gpsimd.dma_start(wqs[d][:], wqT.ap()[ts(d, P), :])
                for d in range(ND):
                    eng = (nc.sync, nc.sync, nc.sync, nc.sync,
                           nc.scalar, nc.scalar, nc.gpsimd, nc.gpsimd)[d]
                    eng.dma_start(qts[d][:], qT.ap()[ts(d, P), :])
                for d in range(ND):
                    nc.sync.dma_start(kts[d][:], kT.ap()[ts(d, P), :])
                for e in range(NE):
                    for qh in range(SQ // 512):
                        ps = psum.tile([P, 512], F32, tag="mm", name=f"ps_q{e}_{qh}")
                        for d in range(ND):
                            nc.tensor.matmul(ps[:], wqs[d][:, ts(e, P)],
                                             qts[d][:, ts(qh, 512)],
                                             start=(d == 0), stop=(d == ND - 1))
                        nc.vector.tensor_copy(QTr[e][:, ts(qh, 512)], ps[:])

            st_res = {}
            # ---- P2+P3 fused: KT chunks feed score matmuls directly ----
            # For each kc (512 keys): project KT[e, kc] for all e, then
            # immediately run the 4 k-tiles of scores using those chunks
            # as lhsT straight from SBUF. No KT spill. V projection (also
            # f32r, reusing the resident kT tiles) follows the score loop
            # in the PE stream so the PE stays busy during the softmax
            # reductions; V spills to DRAM and reloads into freed SBUF.
            with tc.tile_pool(name="wk", bufs=ND) as wk, \
                 tc.tile_pool(name="wvr", bufs=ND) as wvr, \
                 tc.tile_pool(name="ktc", bufs=NE) as ktc, \
                 tc.tile_pool(name="vb", bufs=3) as vb:
                wks = [wk.tile([P, D], F32R, tag="wk", name=f"wk{i}")
                       for i in range(ND)]
                wvs = [wvr.tile([P, D], F32R, tag="wvr", name=f"wvr{i}")
                       for i in range(ND)]
                for d in range(ND):
                    nc.gpsimd.dma_start(wks[d][:], wkT.ap()[ts(d, P), :])
                for d in range(ND):
                    nc.gpsimd.dma_start(wvs[d][:], wvT.ap()[ts(d, P), :])

                for kc in range(S // 512):
                    ktcs = []
                    for e in range(NE):
                        ps = psum.tile([P, 512], F32, tag="mm",
                                       name=f"ps_k{e}_{kc}")
                        for d in range(ND):
                            nc.tensor.matmul(ps[:], wks[d][:, ts(e, P)],
                                             kts[d][:, ts(kc, 512)],
                                             start=(d == 0), stop=(d == ND - 1))
                        kt_c = ktc.tile([P, 512], F32R, tag="ktc",
                                        name=f"ktc{e}_{kc}")
                        nc.vector.tensor_copy(kt_c[:], ps[:])
                        ktcs.append(kt_c)
                    for kk in range(4):          # 4 k-tiles inside this kc
                        k = kc * 4 + kk
                        st_k = stbp.tile([P, SQ], F32, tag="stb", name=f"stb{k}")
                        for qh in range(SQ // 512):
                            ps = psum.tile([P, 512], F32, tag="mm",
                                           name=f"ps_s{k}_{qh}")
                            for e in range(NE):
                                nc.tensor.matmul(ps[:], ktcs[e][:, ts(kk, P)],
                                                 QTr[e][:, ts(qh, 512)],
                                                 start=(e == 0),
                                                 stop=(e == NE - 1))
                            nc.vector.tensor_copy(st_k[:, ts(qh, 512)], ps[:])
                        if k == 0:
                            nc.vector.tensor_copy(macc[:], st_k[:])
                        else:
                            nc.vector.tensor_max(macc[:], macc[:], st_k[:])
                        if k < NK - 2:
                            nc.sync.dma_start(st_spill[k][:], st_k[:])
                        else:
                            st_res[k] = st_k

                # V projection: no softmax deps -> fills PE during reduce
                for k in range(NK):
                    for eh in range(D // 512):
                        ps = psum.tile([P, 512], F32, tag="mm",
                                       name=f"ps_v{k}_{eh}")
                        for d in range(ND):
                            nc.tensor.matmul(ps[:], kts[d][:, ts(k, P)],
                                             wvs[d][:, ts(eh, 512)],
                                             start=(d == 0), stop=(d == ND - 1))
                        v_c = vb.tile([P, 512], BF16, tag="vb",
                                      name=f"vb{k}_{eh}")
                        nc.scalar.copy(v_c[:], ps[:])
                        nc.gpsimd.dma_start(v_spill[k][:, ts(eh, 512)], v_c[:])

            kin_ctx.close()

            # ---- V reload into SBUF freed by the projection inputs ----
            vp = ctx.enter_context(tc.tile_pool(name="vp", bufs=NK))
            V = [vp.tile([P, D], BF16, tag="v", name=f"v{i}") for i in range(NK)]
            for k in range(NK):
                nc.gpsimd.dma_start(V[k][:], v_spill[k][:])

            # ---- P3b: partition halving max -> row max broadcast ------
            # broadcast via DRAM bounce + zero-partition-stride reload:
            # no PE involvement, so it completes under the V matmuls.
            w = 64
            while w >= 1:
                nc.sync.dma_start(tmp[0:w, :], macc[w:2 * w, :])
                nc.vector.tensor_max(macc[0:w, :], macc[0:w, :], tmp[0:w, :])
                w //= 2
            m_dram = dram.tile([1, SQ], F32)
            nc.sync.dma_start(m_dram[:], macc[0:1, :])
            nc.sync.dma_start(m_bc[:], m_dram[0:1, :].to_broadcast([P, SQ]))

            # ---- P4a: E = exp(ST - m) in bf16; accumulate l inline ----
            # ST reloads land in the freed QTr slots (tag reuse) so they
            # can start as soon as the score matmuls finish; the subtract
            # runs in place to avoid extra tiles.
            ep = ctx.enter_context(tc.tile_pool(name="ep", bufs=NK))
            mx2p = ctx.enter_context(tc.tile_pool(name="mx2p", bufs=1))
            lacc = mx2p.tile([P, SQ], F32)
            E = [ep.tile([P, SQ], BF16, tag="e", name=f"e{i}") for i in range(NK)]
            # Half-tile (512-col) sub/exp pipeline: the first PV groups
            # (qc 0..3) only read E columns 0..511, so producing all the
            # first halves before any second half lets PV start after
            # half the serial exp chain.
            korder = [NK - 2, NK - 1] + list(range(NK - 2))
            st_tiles = {}
            for k in korder:
                if k in st_res:
                    st_tiles[k] = st_res[k]
                else:
                    st_k = qtp.tile([P, SQ], F32, tag="qtr",
                                    name=f"stin{k}")
                    eng = nc.sync if k % 2 == 0 else nc.gpsimd
                    eng.dma_start(st_k[:], st_spill[k][:])
                    st_tiles[k] = st_k
            for qh in range(SQ // 512):
                sl = ts(qh, 512)
                for i, k in enumerate(korder):
                    st_k = st_tiles[k]
                    nc.vector.tensor_sub(st_k[:, sl], st_k[:, sl],
                                         m_bc[:, sl])
                    nc.scalar.activation(E[k][:, sl], st_k[:, sl],
                                         mybir.ActivationFunctionType.Exp)
                    if i == 1:
                        nc.vector.tensor_add(lacc[:, sl],
                                             E[korder[0]][:, sl],
                                             E[korder[1]][:, sl])
                    elif i > 1:
                        nc.vector.tensor_add(lacc[:, sl], lacc[:, sl],
                                             E[k][:, sl])

            # ---- P4c: l[q] row sums on DVE; reciprocal to [q,1] layout -
            # lacc = sum over k-tiles of E (fp32 accumulate from bf16),
            # partition-halving sum -> l_row[1, SQ], reciprocal, reshape
            # to [8, 128] via 8 row DMAs, PE-transpose -> recip_t[128, 8];
            # column qc is then the per-partition 1/l for q-chunk qc.
            # ---- P4d: O' = E.T @ V (drains independent of 1/l), then
            # PE-transpose 1/l AFTER the PV matmuls and scale on store ---
            outp = ctx.enter_context(tc.tile_pool(name="outp", bufs=2 * NQC))
            ones_c = consts.tile([P, 1], F32)
            nc.gpsimd.memset(ones_c[:], 1.0)

            # The l row-sum matmuls + PE transpose are threaded into the
            # middle of the PV loop so the whole 1/l path completes while
            # the PE is still doing PV matmuls; drained output chunks are
            # scaled and stored as soon as recip_t exists.
            groups = [(qc, eh) for qc in range(NQC) for eh in range(D // 512)]
            l_row = mx2p.tile([1, SQ], F32)
            r_dram = dram.tile([1, SQ], F32)
            r8 = mx2p.tile([8, P], F32)
            pt8 = psl.tile([P, 8], F32, tag="pt8")
            recip_t = mx2p.tile([P, 8], F32)
            pending = []

            def emit_store(qc, eh, ot, i):
                nc.vector.tensor_scalar_mul(ot[:], ot[:], recip_t[:, qc:qc + 1])
                eng = nc.sync if i % 2 == 0 else nc.scalar
                eng.dma_start(out.ap()[ts(qc, P), ts(eh, 512)], ot[:])

            for g, (qc, eh) in enumerate(groups):
                ps = psum.tile([P, 512], F32, tag="mm", name=f"ps_o{qc}_{eh}")
                for k in range(NK):
                    nc.tensor.matmul(ps[:], E[k][:, ts(qc, P)],
                                     V[k][:, ts(eh, 512)],
                                     start=(k == 0), stop=(k == NK - 1))
                ot = outp.tile([P, 512], F32, tag="ot", name=f"ot{qc}_{eh}")
                nc.vector.tensor_copy(ot[:], ps[:])
                if g < 11:
                    pending.append((qc, eh, ot))
                else:
                    emit_store(qc, eh, ot, g)
                if g == 8:
                    for qh in range(SQ // 512):
                        pl = psl.tile([1, 512], F32, tag="pl", name=f"pl{qh}")
                        nc.tensor.matmul(pl[:], ones_c[:], lacc[:, ts(qh, 512)],
                                         start=True, stop=True)
                        nc.vector.tensor_copy(l_row[0:1, ts(qh, 512)], pl[:])
                    nc.sync.dma_start(r_dram[:], l_row[:])
                    nc.sync.dma_start(
                        r8[:], r_dram[0, :].rearrange("(a b) -> a b", a=8))
                elif g == 10:
                    nc.tensor.transpose(pt8[:], r8[:], id8[:])
                    nc.vector.reciprocal(recip_t[:], pt8[:])
                    for i, (pqc, peh, pot) in enumerate(pending):
                        emit_store(pqc, peh, pot, i)

    nc.compile()
    _NC_CACHE["nc"] = nc
    return nc


def kernel(query, key, Wq, Wk, Wv):
    query = np.asarray(query, dtype=np.float32)
    key = np.asarray(key, dtype=np.float32)
    wqT = np.ascontiguousarray(np.asarray(Wq, dtype=np.float32).T)
    wkT = np.ascontiguousarray(np.asarray(Wk, dtype=np.float32).T)
    wvT_np = np.ascontiguousarray(np.asarray(Wv, dtype=np.float32).T)

    in_maps = []
    for c in range(N_CORES):
        b, h = c // 2, c % 2
        qTn = np.ascontiguousarray(query[b, h * SQ:(h + 1) * SQ, :].T)
        kTn = np.ascontiguousarray(key[b].T)
        in_maps.append({
            "qT": qTn, "kT": kTn, "wqT": wqT, "wkT": wkT, "wvT": wvT_np,
        })

    nc = _build()
    res = run_bass_kernel_spmd(nc, in_maps, core_ids=list(range(N_CORES)))
    outv = np.empty((B, S, D), dtype=np.float32)
    for c in range(N_CORES):
        b, h = c // 2, c % 2
        outv[b, h * SQ:(h + 1) * SQ, :] = res.results[c]["out"]
    return outv


# revision 38
# speedup vs baseline: 1.0463x; 1.0463x over previous
"""Trainium2 Bass kernel: attention layer (B=4, S=2048, D=1024), 8 NeuronCores.

Sharding: data-parallel over (batch, query-half) -> 8 shards. Each core
computes one batch's half of the queries against that batch's full key/value
(tensor-parallel K/V splits were measured and rejected: a pairwise 4MB
AllGather costs >150us on this part, far more than the 60us of matmuls
it would save).

Per-core dataflow (all transpose-free; host pre-transposes inputs):
  QT[e,q]   = Wq projection of queries (fp32r matmuls; stays resident)
  KT chunks = Wk projection, fused with the score phase: each [e,512k]
              drain chunk is consumed as score-matmul lhsT straight from
              SBUF, so KT is never materialized or spilled
  ST[k,q]   = scores, k on partitions (fp32r, fp32 PSUM); spilled to DRAM
              except the last two k-tiles; row-max tracked on the fly by
              a DVE max chain
  softmax along k (the partition dim): 7-step DMA-shift partition-halving
    max -> m[1,q]; broadcast to [128,q] via DRAM bounce + zero-partition-
    stride reload; E = exp(ST - m) in bf16 (ST reloads land in the freed
    QT slots; subtract runs in place); l accumulated on DVE as E appears
  V[k,e]    = value projection (fp32r from the resident key tiles),
              emitted after the scores so the PE stays busy through the
              softmax reductions; spills to DRAM, reloads into freed SBUF
  O[q,e]    = (E.T @ V) matmuls (bf16); the l row-sum ones-matmuls, the
              PE-transpose of 1/l into [q,1] layout, and the scaled
              stores are threaded into the middle of the PV loop
Engine budget at ~330us: PE ~281us busy (84%), DVE ~137, ACT ~49,
gpsimd ~62, sync ~36; MFU ~72%.

float32r matmul measured on silicon: ~1 cycle/row at free-dim 512 (vs
fp32's 4) with ~1.5e-4 relative precision -- enough for the unscaled
(near-one-hot, logit std ~34) softmax here; bf16 scores would flip
argmax rows and fail (measured 2.5e-2). bf16 is fine for E and the PV
contraction. End-to-end rel err vs the fp32 reference: 2.3e-3.
"""

import numpy as np
import ml_dtypes
from contextlib import ExitStack

import concourse.bass as bass
import concourse.tile as tile
from concourse import bacc, mybir
from concourse.bass import ts
from concourse.bass_utils import run_bass_kernel_spmd

B, S, D = 4, 2048, 1024
N_CORES = 8
SQ = S // 2            # 1024 query rows per core
P = 128                # partitions
NE = D // P            # 8 e-tiles
ND = D // P            # 8 d-tiles
NK = S // P            # 16 k-tiles
NQC = SQ // P          # 8 q-chunks
F32R = mybir.dt.float32r
F32 = mybir.dt.float32
BF16 = mybir.dt.bfloat16

_NC_CACHE = {}


def _build():
    if "nc" in _NC_CACHE:
        return _NC_CACHE["nc"]
    nc = bacc.Bacc("TRN2", target_bir_lowering=False, debug=False,
                   num_devices=N_CORES)

    qT = nc.dram_tensor("qT", [D, SQ], F32R, kind="ExternalInput")
    kT = nc.dram_tensor("kT", [D, S], F32R, kind="ExternalInput")
    wqT = nc.dram_tensor("wqT", [D, D], F32R, kind="ExternalInput")
    wkT = nc.dram_tensor("wkT", [D, D], F32R, kind="ExternalInput")
    wvT = nc.dram_tensor("wvT", [D, D], F32R, kind="ExternalInput")
    out = nc.dram_tensor("out", [SQ, D], F32, kind="ExternalOutput")

    from concourse.masks import make_identity

    with tile.TileContext(nc) as tc:
        with ExitStack() as ctx:
            psum = ctx.enter_context(tc.tile_pool(name="psum", bufs=6, space="PSUM"))
            psl = ctx.enter_context(tc.tile_pool(name="psl", bufs=1, space="PSUM"))
            dram = ctx.enter_context(tc.tile_pool(name="dram", bufs=1, space="DRAM"))
            consts = ctx.enter_context(tc.tile_pool(name="consts", bufs=1))
            maxp = ctx.enter_context(tc.tile_pool(name="maxp", bufs=1))
            qtp = ctx.enter_context(tc.tile_pool(name="qtp", bufs=NE))

            id8 = consts.tile([8, 8], F32)
            make_identity(nc, id8[:])

            macc = maxp.tile([P, SQ], F32)
            stbp = ctx.enter_context(tc.tile_pool(name="stbp", bufs=2))
            tmp = maxp.tile([64, SQ], F32)
            m_bc = maxp.tile([P, SQ], F32)

            st_spill = [dram.tile([P, SQ], F32, tag="stsp", name=f"stsp{i}")
                        for i in range(NK)]
            v_spill = [dram.tile([P, D], BF16, tag="vsp", name=f"vsp{i}")
                       for i in range(NK)]

            # ---- P1: QT[e,q] projection; stays resident in SBUF -------
            # kin (the f32r key tiles) opens early so its 8MB load runs
            # on the sync queue during the P1 matmuls.
            kin_ctx = ExitStack()
            kin = kin_ctx.enter_context(tc.tile_pool(name="kin", bufs=ND))
            kts = [kin.tile([P, S], F32R, tag="kin", name=f"kin{i}")
                   for i in range(ND)]
            QTr = [qtp.tile([P, SQ], F32R, tag="qtr", name=f"qtr{i}")
                   for i in range(NE)]
            with tc.tile_pool(name="qin", bufs=ND) as qin, \
                 tc.tile_pool(name="wq", bufs=ND) as wq:
                qts = [qin.tile([P, SQ], F32R, tag="qin", name=f"qin{i}")
                       for i in range(ND)]
                wqs = [wq.tile([P, D], F32R, tag="wq", name=f"wq{i}")
                       for i in range(ND)]
                for d in range(ND):
                    nc.gpsimd.dma_start(wqs[d][:], wqT.ap()[ts(d, P), :])
                for d in range(ND):
                    eng = nc.sync if d < 6 else nc.scalar
                    eng.dma_start(qts[d][:], qT.ap()[ts(d, P), :])
                for d in range(ND):
                    nc.sync.dma_start(kts[d][:], kT.ap()[ts(d, P), :])
                for e in range(NE):
                    for qh in range(SQ // 512):
                        ps = psum.tile([P, 512], F32, tag="mm", name=f"ps_q{e}_{qh}")
                        for d in range(ND):
                            nc.tensor.matmul(ps[:], wqs[d][:, ts(e, P)],
                                             qts[d][:, ts(qh, 512)],
                                             start=(d == 0), stop=(d == ND - 1))
                        nc.vector.tensor_copy(QTr[e][:, ts(qh, 512)], ps[:])

            st_res = {}
            # ---- P2+P3 fused: KT chunks feed score matmuls directly ----
            # For each kc (512 keys): project KT[e, kc] for all e, then
            # immediately run the 4 k-tiles of scores using those chunks
            # as lhsT straight from SBUF. No KT spill. V projection (also
            # f32r, reusing the resident kT tiles) follows the score loop
            # in the PE stream so the PE stays busy during the softmax
            # reductions; V spills to DRAM and reloads into freed SBUF.
            with tc.tile_pool(name="wk", bufs=ND) as wk, \
                 tc.tile_pool(name="wvr", bufs=ND) as wvr, \
                 tc.tile_pool(name="ktc", bufs=NE) as ktc, \
                 tc.tile_pool(name="vb", bufs=3) as vb:
                wks = [wk.tile([P, D], F32R, tag="wk", name=f"wk{i}")
                       for i in range(ND)]
                wvs = [wvr.tile([P, D], F32R, tag="wvr", name=f"wvr{i}")
                       for i in range(ND)]
                for d in range(ND):
                    nc.gpsimd.dma_start(wks[d][:], wkT.ap()[ts(d, P), :])
                for d in range(ND):
                    nc.gpsimd.dma_start(wvs[d][:], wvT.ap()[ts(d, P), :])

                for kc in range(S // 512):
                    ktcs = []
                    for e in range(NE):
                        ps = psum.tile([P, 512], F32, tag="mm",
                                       name=f"ps_k{e}_{kc}")
                        for d in range(ND):
                            nc.tensor.matmul(ps[:], wks[d][:, ts(e, P)],
                                             kts[d][:, ts(kc, 512)],
                                             start=(d == 0), stop=(d == ND - 1))
                        kt_c = ktc.tile([P, 512], F32R, tag="ktc",
                                        name=f"ktc{e}_{kc}")
                        nc.vector.tensor_copy(kt_c[:], ps[:])
                        ktcs.append(kt_c)
                    for kk in range(4):          # 4 k-tiles inside this kc
                        k = kc * 4 + kk
                        st_k = stbp.tile([P, SQ], F32, tag="stb", name=f"stb{k}")
                        for qh in range(SQ // 512):
                            ps = psum.tile([P, 512], F32, tag="mm",
                                           name=f"ps_s{k}_{qh}")
                            for e in range(NE):
                                nc.tensor.matmul(ps[:], ktcs[e][:, ts(kk, P)],
                                                 QTr[e][:, ts(qh, 512)],
                                                 start=(e == 0),
                                                 stop=(e == NE - 1))
                            nc.vector.tensor_copy(st_k[:, ts(qh, 512)], ps[:])
                        if k == 0:
                            nc.vector.tensor_copy(macc[:], st_k[:])
                        else:
                            nc.vector.tensor_max(macc[:], macc[:], st_k[:])
                        if k < NK - 2:
                            nc.sync.dma_start(st_spill[k][:], st_k[:])
                        else:
                            st_res[k] = st_k

                # V projection: no softmax deps -> fills PE during reduce
                for k in range(NK):
                    for eh in range(D // 512):
                        ps = psum.tile([P, 512], F32, tag="mm",
                                       name=f"ps_v{k}_{eh}")
                        for d in range(ND):
                            nc.tensor.matmul(ps[:], kts[d][:, ts(k, P)],
                                             wvs[d][:, ts(eh, 512)],
                                             start=(d == 0), stop=(d == ND - 1))
                        v_c = vb.tile([P, 512], BF16, tag="vb",
                                      name=f"vb{k}_{eh}")
                        nc.scalar.copy(v_c[:], ps[:])
                        nc.gpsimd.dma_start(v_spill[k][:, ts(eh, 512)], v_c[:])

            kin_ctx.close()

            # ---- V reload into SBUF freed by the projection inputs ----
            vp = ctx.enter_context(tc.tile_pool(name="vp", bufs=NK))
            V = [vp.tile([P, D], BF16, tag="v", name=f"v{i}") for i in range(NK)]
            for k in range(NK):
                nc.gpsimd.dma_start(V[k][:], v_spill[k][:])

            # ---- P3b: partition halving max -> row max broadcast ------
            # broadcast via DRAM bounce + zero-partition-stride reload:
            # no PE involvement, so it completes under the V matmuls.
            w = 64
            while w >= 1:
                nc.sync.dma_start(tmp[0:w, :], macc[w:2 * w, :])
                nc.vector.tensor_max(macc[0:w, :], macc[0:w, :], tmp[0:w, :])
                w //= 2
            m_dram = dram.tile([1, SQ], F32)
            nc.sync.dma_start(m_dram[:], macc[0:1, :])
            nc.sync.dma_start(m_bc[:], m_dram[0:1, :].to_broadcast([P, SQ]))

            # ---- P4a: E = exp(ST - m) in bf16; accumulate l inline ----
            # ST reloads land in the freed QTr slots (tag reuse) so they
            # can start as soon as the score matmuls finish; the subtract
            # runs in place to avoid extra tiles.
            ep = ctx.enter_context(tc.tile_pool(name="ep", bufs=NK))
            mx2p = ctx.enter_context(tc.tile_pool(name="mx2p", bufs=1))
            lacc = mx2p.tile([P, SQ], F32)
            E = [ep.tile([P, SQ], BF16, tag="e", name=f"e{i}") for i in range(NK)]
            # Half-tile (512-col) sub/exp pipeline: the first PV groups
            # (qc 0..3) only read E columns 0..511, so producing all the
            # first halves before any second half lets PV start after
            # half the serial exp chain.
            korder = [NK - 2, NK - 1] + list(range(NK - 2))
            st_tiles = {}
            for k in korder:
                if k in st_res:
                    st_tiles[k] = st_res[k]
                else:
                    st_k = qtp.tile([P, SQ], F32, tag="qtr",
                                    name=f"stin{k}")
                    eng = nc.sync if k % 2 == 0 else nc.gpsimd
                    eng.dma_start(st_k[:], st_spill[k][:])
                    st_tiles[k] = st_k
            for qh in range(SQ // 512):
                sl = ts(qh, 512)
                for i, k in enumerate(korder):
                    st_k = st_tiles[k]
                    nc.vector.tensor_sub(st_k[:, sl], st_k[:, sl],
                                         m_bc[:, sl])
                    nc.scalar.activation(E[k][:, sl], st_k[:, sl],
                                         mybir.ActivationFunctionType.Exp)
                    if i == 1:
                        nc.vector.tensor_add(lacc[:, sl],
                                             E[korder[0]][:, sl],
                                             E[korder[1]][:, sl])
                    elif i > 1:
                        nc.vector.tensor_add(lacc[:, sl], lacc[:, sl],
                                             E[k][:, sl])

            # ---- P4c: l[q] row sums on DVE; reciprocal to [q,1] layout -
            # lacc = sum over k-tiles of E (fp32 accumulate from bf16),
            # partition-halving sum -> l_row[1, SQ], reciprocal, reshape
            # to [8, 128] via 8 row DMAs, PE-transpose -> recip_t[128, 8];
            # column qc is then the per-partition 1/l for q-chunk qc.
            # ---- P4d: O' = E.T @ V (drains independent of 1/l), then
            # PE-transpose 1/l AFTER the PV matmuls and scale on store ---
            outp = ctx.enter_context(tc.tile_pool(name="outp", bufs=2 * NQC))
            ones_c = consts.tile([P, 1], F32)
            nc.gpsimd.memset(ones_c[:], 1.0)

            # The l row-sum matmuls + PE transpose are threaded into the
            # middle of the PV loop so the whole 1/l path completes while
            # the PE is still doing PV matmuls; drained output chunks are
            # scaled and stored as soon as recip_t exists.
            groups = [(qc, eh) for qc in range(NQC) for eh in range(D // 512)]
            l_row = mx2p.tile([1, SQ], F32)
            r_dram = dram.tile([1, SQ], F32)
            r8 = mx2p.tile([8, P], F32)
            pt8 = psl.tile([P, 8], F32, tag="pt8")
            recip_t = mx2p.tile([P, 8], F32)
            pending = []

            def emit_store(qc, eh, ot, i):
                nc.vector.tensor_scalar_mul(ot[:], ot[:], recip_t[:, qc:qc + 1])
                eng = nc.sync if i % 2 == 0 else nc.scalar
                eng.dma_start(out.ap()[ts(qc, P), ts(eh, 512)], ot[:])

            for g, (qc, eh) in enumerate(groups):
                ps = psum.tile([P, 512], F32, tag="mm", name=f"ps_o{qc}_{eh}")
                for k in range(NK):
                    nc.tensor.matmul(ps[:], E[k][:, ts(qc, P)],
                                     V[k][:, ts(eh, 512)],
                                     start=(k == 0), stop=(k == NK - 1))
                ot = outp.tile([P, 512], F32, tag="ot", name=f"ot{qc}_{eh}")
                nc.vector.tensor_copy(ot[:], ps[:])
                if g < 11:
                    pending.append((qc, eh, ot))
                else:
                    emit_store(qc, eh, ot, g)
                if g == 8:
                    for qh in range(SQ // 512):
                        pl = psl.tile([1, 512], F32, tag="pl", name=f"pl{qh}")
                        nc.tensor.matmul(pl[:], ones_c[:], lacc[:, ts(qh, 512)],
                                         start=True, stop=True)
                        nc.vector.tensor_copy(l_row[0:1, ts(qh, 512)], pl[:])
                    nc.sync.dma_start(r_dram[:], l_row[:])
                    nc.sync.dma_start(
                        r8[:], r_dram[0, :].rearrange("(a b) -> a b", a=8))
                elif g == 10:
                    nc.tensor.transpose(pt8[:], r8[:], id8[:])
                    nc.vector.reciprocal(recip_t[:], pt8[:])
                    for i, (pqc, peh, pot) in enumerate(pending):
                        emit_store(pqc, peh, pot, i)

    nc.compile()
    _NC_CACHE["nc"] = nc
    return nc


def kernel(query, key, Wq, Wk, Wv):
    query = np.asarray(query, dtype=np.float32)
    key = np.asarray(key, dtype=np.float32)
    wqT = np.ascontiguousarray(np.asarray(Wq, dtype=np.float32).T)
    wkT = np.ascontiguousarray(np.asarray(Wk, dtype=np.float32).T)
    wvT_np = np.ascontiguousarray(np.asarray(Wv, dtype=np.float32).T)

    in_maps = []
    for c in range(N_CORES):
        b, h = c // 2, c % 2
        qTn = np.ascontiguousarray(query[b, h * SQ:(h + 1) * SQ, :].T)
        kTn = np.ascontiguousarray(key[b].T)
        in_maps.append({
            "qT": qTn, "kT": kTn, "wqT": wqT, "wkT": wkT, "wvT": wvT_np,
        })

    nc = _build()
    res = run_bass_kernel_spmd(nc, in_maps, core_ids=list(range(N_CORES)))
    outv = np.empty((B, S, D), dtype=np.float32)
    for c in range(N_CORES):
        b, h = c // 2, c % 2
        outv[b, h * SQ:(h + 1) * SQ, :] = res.results[c]["out"]
    return outv
